# revision 11
# baseline (speedup 1.0000x reference)
"""Trainium2 Bass kernel for nn_AltInnerProductDecoder.

Computes, for all upper-triangular pairs (i<j) of N=2048 nodes:
    f    = concat(z[i]*z[j], j-i)                  # 65 features
    y    = LayerNorm(f) * gamma + beta
    h    = relu(y @ w1 + b1)                       # 32 hidden
    out  = h @ w2 + b2                             # scalar per pair
scattered into a dense [N, N] upper-triangular adjacency.

Strategy: the per-pair 65->32 linear layer is a set of 32 weighted Gram
matrices of z, computed as dense [128 x 512] output tiles on the
TensorEngine (float32r, 1 cycle/row).  LayerNorm folds:
  out(i,j) = rstd * ( sum_k w2_k * relu( G'_k + b1'_k * u ) + b2 * u )
with u = std = sqrt(var+eps), rstd = 1/u (rstd*u = 1 kills b2's u).
G'_k is the mean-corrected Gram (correction folded into weights on host).
The distance feature d = j-i is folded into extra contraction rows using
bf16-exact hi/lo splits so reduced-precision matmul modes stay accurate.

8 cores, SPMD: 40 upper-triangle tiles of [128 x 512] split 5-per-core
(core c owns i-tile-rows c and 15-c).  All per-core variation is input
data (same graph).  Host assembles + masks with np.triu(k=1).
"""

import os
import sys

import numpy as np

sys.path.insert(0, "/opt/trn_rl_repo")

N = 2048
L = 64
D = 65  # input features to MLP
H = 32  # hidden
EPS = 1e-5
TI = 128  # i-tile rows
TJ = 512  # j-tile cols
NB_I = N // TI  # 16
NB_J = N // TJ  # 4
NCORES = 8
TILES_PER_CORE = 5

K1 = 72  # contraction rows for mov1/statG/statS0
K2 = 75  # contraction rows for mov2/statS1

# DVE handles k's with k%8<5 (20 of 32, fused relu+acc chain);
# ACT handles the rest (12, relu -> PE acc matmul).
DVE_KS = [k for k in range(H) if k % 8 < 5]
ACT_KS = [k for k in range(H) if k % 8 >= 5]


def core_tiles(c: int) -> list[tuple[int, int]]:
    """5 (bi, bj) tiles for core c covering the upper triangle."""
    tiles = []
    for bi in (c, NB_I - 1 - c):
        bj_min = (TI * bi + 1) // TJ
        for bj in range(bj_min, NB_J):
            tiles.append((bi, bj))
    assert len(tiles) == TILES_PER_CORE, (c, tiles)
    return tiles


def _bf16_round(x: np.ndarray) -> np.ndarray:
    import ml_dtypes

    return x.astype(np.float32).astype(ml_dtypes.bfloat16).astype(np.float64)


def _split3(x: np.ndarray):
    """3-way bf16-exact split: x ~= h + m + l with each part bf16-representable."""
    h = _bf16_round(x)
    r = x - h
    m = _bf16_round(r)
    l = _bf16_round(r - m)
    return h, m, l


def _split2(x: np.ndarray):
    h = _bf16_round(x)
    l = _bf16_round(x - h)
    return h, l


def host_precompute(z, w1, b1, w2, b2, gamma, beta):
    """Returns per-core input maps (list of dicts of np.float32 arrays) and
    the folded scalars needed at graph-build time."""
    z = np.asarray(z, np.float64)
    w1 = np.asarray(w1, np.float64)
    b1 = np.asarray(b1, np.float64).reshape(H)
    w2 = np.asarray(w2, np.float64).reshape(H)
    b2 = float(np.asarray(b2, np.float64).reshape(()))
    gamma = np.asarray(gamma, np.float64).reshape(D)
    beta = np.asarray(beta, np.float64).reshape(D)

    wg = gamma[:, None] * w1  # [65, 32]
    ck = wg.sum(axis=0)  # [32]
    wg2 = wg - ck[None, :] / D  # [65, 32] mean-corrected
    b1p = b1 + beta @ w1  # [32]

    sc = np.where(w2 >= 0, np.abs(w2), -np.abs(w2))  # = w2 (signed scale)

    z2 = z * z

    in_maps = []
    for c in range(NCORES):
        tiles = core_tiles(c)
        statG = np.zeros((TILES_PER_CORE, K1, H + 1, TI), np.float64)
        statS1 = np.zeros((TILES_PER_CORE, K2, TI), np.float64)
        mov1 = np.zeros((TILES_PER_CORE, K1, TJ), np.float64)
        mov2 = np.zeros((TILES_PER_CORE, K2, TJ), np.float64)

        for t, (bi, bj) in enumerate(tiles):
            i0, j0 = bi * TI, bj * TJ
            icc = np.arange(TI, dtype=np.float64) - (TI - 1) / 2.0  # [-63.5..63.5]
            jcc = np.arange(TJ, dtype=np.float64) - (TJ - 1) / 2.0  # [-255.5..255.5]
            D0 = (j0 + (TJ - 1) / 2.0) - (i0 + (TI - 1) / 2.0)  # d = D0 - icc + jcc
            r_i = D0 - icc  # [TI], d(i,j) = r_i + jcc[j]

            zi = z[i0 : i0 + TI]  # [TI, L]
            zj = z[j0 : j0 + TJ]  # [TJ, L]

            jh, jl = _split2(jcc)
            jsq3 = _split3(jcc * jcc)

            # ---- mov1 [K1, TJ]: rows 0-63 z_j; 64-66 ones; 67 jh; 68 jl;
            #      69 jh; 70-71 split2(jcc/65) (for S0, stat=1 exact)
            mov1[t, :L] = zj.T
            mov1[t, L : L + 3] = 1.0
            mov1[t, 67] = jh
            mov1[t, 68] = jl
            mov1[t, 69] = jh
            j65h, j65l = _split2(jcc / D)
            mov1[t, 70] = j65h
            mov1[t, 71] = j65l

            # ---- mov2 [K2, TJ]: rows 0-63 z_j^2; 64-66 ones; 67-69 jh;
            #      70-71 jl; 72-74 split3(jcc^2/65) (stat=1 exact)
            mov2[t, :L] = (zj * zj).T
            mov2[t, L : L + 3] = 1.0
            mov2[t, 67:70] = jh
            mov2[t, 70:72] = jl
            jsq65_3 = _split3(jcc * jcc / D)
            for r in range(3):
                mov2[t, 72 + r] = jsq65_3[r]

            # ---- statG for k in 0..31 (scaled by sc_k) and S0 at index 32
            for k in range(H):
                s = sc[k]
                statG[t, :L, k] = (zi * (wg2[:L, k] * s)).T  # [L, TI]
                q = (wg2[L, k] * s) * r_i  # [TI]
                q3 = _split3(q)
                for r in range(3):
                    statG[t, L + r, k] = q3[r]
                wh, wl = _split2(np.asarray(wg2[L, k] * s))
                statG[t, 67, k] = wh  # pairs mov1 jh
                statG[t, 68, k] = wh  # pairs mov1 jl
                statG[t, 69, k] = wl  # pairs mov1 jh
            # S0 (mean = m0/65): stationary scaled by 1/65
            statG[t, :L, H] = (zi / D).T
            r3 = _split3(r_i / D)
            for r in range(3):
                statG[t, L + r, H] = r3[r]
            statG[t, 70, H] = 1.0
            statG[t, 71, H] = 1.0

            # ---- statS1 (m1/65): rows 0-63 z_i^2/65; 64-66 r_i^2/65 splits;
            #      67-69 (2 r_i/65) 3-split x jh; 70-71 first two x jl; 72-74 1/65
            statS1[t, :L] = (zi * zi / D).T
            r2_3 = _split3(r_i * r_i / D)
            for r in range(3):
                statS1[t, L + r] = r2_3[r]
            s3 = _split3(2.0 * r_i / D)
            for r in range(3):
                statS1[t, 67 + r] = s3[r]
            statS1[t, 70] = s3[0]
            statS1[t, 71] = s3[1]
            statS1[t, 72:75] = 1.0

        # identities: k -> b1p[k]*sc[k]*I ; 32 -> b2*I ; 33 -> +I ; 34 -> -I
        ids = np.zeros((TI, H + 3, TI), np.float64)
        eye = np.eye(TI)
        for k in range(H):
            ids[:, k, :] = (b1p[k] * sc[k]) * eye
        ids[:, H, :] = b2 * eye
        ids[:, H + 1, :] = eye
        ids[:, H + 2, :] = -eye

        import ml_dtypes

        bf16 = ml_dtypes.bfloat16
        in_maps.append(
            {
                "statg": np.ascontiguousarray(statG.astype(np.float32), bf16).reshape(
                    TILES_PER_CORE, K1, (H + 1) * TI
                ),
                "stats1": np.ascontiguousarray(statS1.astype(np.float32), bf16),
                "mov1": np.ascontiguousarray(mov1.astype(np.float32), bf16),
                "mov2": np.ascontiguousarray(mov2.astype(np.float32), bf16),
                "ids": np.ascontiguousarray(ids.astype(np.float32), bf16),
            }
        )

    w2_signs = tuple(bool(s >= 0) for s in w2)
    return in_maps, w2_signs


def build_graph(w2_signs):
    """Build the SPMD Bacc graph (same for all cores)."""
    from concourse import bacc, mybir
    import concourse.bass as bass
    import concourse.tile as tile

    f32 = mybir.dt.float32
    bf16 = mybir.dt.bfloat16
    AF = mybir.ActivationFunctionType
    ALU = mybir.AluOpType

    nc = bacc.Bacc("TRN2", target_bir_lowering=False, debug=False, num_devices=NCORES)

    statg_d = nc.declare_dram_parameter(
        "statg", [TILES_PER_CORE, K1, (H + 1) * TI], bf16, isOutput=False
    )
    stats1_d = nc.declare_dram_parameter(
        "stats1", [TILES_PER_CORE, K2, TI], bf16, isOutput=False
    )
    mov1_d = nc.declare_dram_parameter(
        "mov1", [TILES_PER_CORE, K1, TJ], bf16, isOutput=False
    )
    mov2_d = nc.declare_dram_parameter(
        "mov2", [TILES_PER_CORE, K2, TJ], bf16, isOutput=False
    )
    ids_d = nc.declare_dram_parameter("ids", [TI, H + 3, TI], bf16, isOutput=False)
    out_d = nc.declare_dram_parameter(
        "out", [TILES_PER_CORE, TI, TJ], f32, isOutput=True
    )

    with tile.TileContext(nc) as tc:
        with (
            tc.tile_pool(name="consts", bufs=1) as consts,
            tc.tile_pool(name="statg", bufs=2) as statg_p,
            tc.tile_pool(name="stats1", bufs=2) as stats1_p,
            tc.tile_pool(name="mov", bufs=2) as mov_p,
            tc.tile_pool(name="chain", bufs=3) as chain_p,
            tc.tile_pool(name="work", bufs=2) as work_p,
            tc.tile_pool(name="hrelu", bufs=3) as hrelu_p,
            tc.tile_pool(name="ps_s0", bufs=2, space="PSUM") as ps_s0,
            tc.tile_pool(name="ps_s1", bufs=2, space="PSUM") as ps_s1,
            tc.tile_pool(name="ps_acc", bufs=2, space="PSUM") as ps_acc,
            tc.tile_pool(name="ps_g", bufs=2, space="PSUM") as ps_g,
        ):
            ids_sb = consts.tile([TI, H + 3, TI], bf16)
            nc.sync.dma_start(ids_sb[:], ids_d[:])
            eps_sb = consts.tile([TI, 1], f32)
            nc.vector.memset(eps_sb[:], EPS)

            state = {}

            def prologue(t):
                sg = statg_p.tile([K1, H + 1, TI], bf16, tag="sg")
                nc.sync.dma_start(sg[:], statg_d[t].rearrange("k (a p) -> k a p", p=TI))
                s1t = stats1_p.tile([K2, TI], bf16, tag="s1t")
                nc.sync.dma_start(s1t[:], stats1_d[t])
                m1 = mov_p.tile([K1, TJ], bf16, tag="m1")
                nc.sync.dma_start(m1[:], mov1_d[t])
                m2 = mov_p.tile([K2, TJ], bf16, tag="m2")
                nc.sync.dma_start(m2[:], mov2_d[t])

                s0_ps = ps_s0.tile([TI, TJ], f32, tag="s0ps")
                s1_ps = ps_s1.tile([TI, TJ], f32, tag="s1ps")
                nc.tensor.matmul(s0_ps[:], sg[:, H, :], m1[:])
                nc.tensor.matmul(s1_ps[:], s1t[:], m2[:])

                mean_sb = work_p.tile([TI, TJ], f32, tag="mean")
                nc.scalar.copy(mean_sb[:], s0_ps[:])
                q = work_p.tile([TI, TJ], f32, tag="q")
                nc.vector.tensor_mul(q[:], mean_sb[:], mean_sb[:])  # mean^2
                var = work_p.tile([TI, TJ], f32, tag="var")
                nc.vector.tensor_sub(var[:], s1_ps[:], q[:])
                lnv = work_p.tile([TI, TJ], f32, tag="lnv")
                nc.scalar.activation(lnv[:], var[:], AF.Ln, bias=eps_sb[:])
                u = work_p.tile([TI, TJ], bf16, tag="u")
                nc.scalar.activation(u[:], lnv[:], AF.Exp, scale=0.5)
                rstd = work_p.tile([TI, TJ], f32, tag="rstd")
                nc.scalar.activation(rstd[:], lnv[:], AF.Exp, scale=-0.5)
                state[t] = (sg, m1, u, rstd)

            def kloop(t):
                sg, m1, u, rstd = state.pop(t)
                acc_ps = ps_acc.tile([TI, TJ], f32, tag="accps")
                # ACC = b2 * u
                nc.tensor.matmul(
                    acc_ps[:], ids_sb[:, H, :], u[:], start=True, stop=False
                )
                chain_prev = None
                n_act_done = 0
                for k in range(H):
                    g_ps = ps_g.tile([TI, TJ], f32, tag="gps")
                    nc.tensor.matmul(
                        g_ps[:], sg[:, k, :], m1[:], start=True, stop=False
                    )
                    nc.tensor.matmul(
                        g_ps[:], ids_sb[:, k, :], u[:], start=False, stop=True
                    )
                    pos = w2_signs[k]
                    if k % 8 < 5:  # DVE: fused relu + accumulate chain
                        op0 = ALU.max if pos else ALU.min
                        cnew = chain_p.tile([TI, TJ], f32, tag="chain")
                        if chain_prev is None:
                            nc.vector.tensor_scalar(
                                cnew[:], g_ps[:], 0.0, None, op0
                            )
                        else:
                            nc.vector.scalar_tensor_tensor(
                                cnew[:], g_ps[:], 0.0, chain_prev[:], op0, ALU.add
                            )
                        chain_prev = cnew
                    else:  # ACT relu -> PE accumulate
                        h = hrelu_p.tile([TI, TJ], bf16, tag="h")
                        nc.scalar.activation(
                            h[:], g_ps[:], AF.Relu, scale=1.0 if pos else -1.0
                        )
                        n_act_done += 1
                        last = n_act_done == len(ACT_KS)
                        idm = H + 1 if pos else H + 2
                        nc.tensor.matmul(
                            acc_ps[:],
                            ids_sb[:, idm, :],
                            h[:],
                            start=False,
                            stop=last,
                        )
                # out = rstd * (ACC + chain)
                tsum = work_p.tile([TI, TJ], f32, tag="tsum")
                nc.vector.scalar_tensor_tensor(
                    tsum[:], acc_ps[:], 1.0, chain_prev[:], ALU.mult, ALU.add
                )
                out_sb = work_p.tile([TI, TJ], f32, tag="outsb")
                nc.vector.tensor_mul(out_sb[:], tsum[:], rstd[:])
                nc.sync.dma_start(out_d[t], out_sb[:])

            prologue(0)
            for t in range(TILES_PER_CORE):
                if t + 1 < TILES_PER_CORE:
                    prologue(t + 1)
                kloop(t)

    nc.compile()
    return nc


_CACHE = {}


def _make_runner(nc):
    """Build a cached jitted SPMD runner (mirrors bass2jax.run_bass_via_pjrt
    but keeps the compiled executable across calls)."""
    import jax
    from jax.experimental.shard_map import shard_map
    from jax.sharding import Mesh, PartitionSpec
    from concourse import bass2jax as b2j
    from concourse import mybir

    b2j.install_neuronx_cc_hook()

    partition_name = nc.partition_id_tensor.name if nc.partition_id_tensor else None
    in_names, out_names, out_avals, zero_outs = [], [], [], []
    for alloc in nc.m.functions[0].allocations:
        if not isinstance(alloc, mybir.MemoryLocationSet):
            continue
        name = alloc.memorylocations[0].name
        if alloc.kind == "ExternalInput":
            if name != partition_name:
                in_names.append(name)
        elif alloc.kind == "ExternalOutput":
            out_names.append(name)
            shape = tuple(alloc.tensor_shape)
            dtype = mybir.dt.np(alloc.dtype)
            out_avals.append(jax.core.ShapedArray(shape, dtype))
            zero_outs.append(np.zeros(shape, dtype))
    n_params = len(in_names)
    all_in_names = list(in_names) + list(out_names)
    if partition_name is not None:
        all_in_names.append(partition_name)

    def _body(*args):
        operands = list(args)
        if partition_name is not None:
            operands.append(b2j.partition_id_tensor())
        outs = b2j._bass_exec_p.bind(
            *operands,
            out_avals=tuple(out_avals),
            in_names=tuple(all_in_names),
            out_names=tuple(out_names),
            lowering_input_output_aliases=(),
            sim_require_finite=False,
            sim_require_nnan=False,
            nc=nc,
        )
        return tuple(outs)

    devices = jax.devices()[:NCORES]
    mesh = Mesh(np.asarray(devices), ("core",))
    n_outs = len(out_names)
    in_specs = (PartitionSpec("core"),) * (n_params + n_outs)
    out_specs = (PartitionSpec("core"),) * n_outs
    sharded = jax.jit(
        shard_map(
            _body, mesh=mesh, in_specs=in_specs, out_specs=out_specs, check_rep=False
        ),
        keep_unused=True,
    )
    concat_zeros = [
        np.zeros((NCORES * z.shape[0], *z.shape[1:]), z.dtype) for z in zero_outs
    ]

    def run(in_maps):
        concat_in = [
            np.concatenate([in_maps[c][name] for c in range(NCORES)], axis=0)
            for name in in_names
        ]
        out_arrs = sharded(*concat_in, *concat_zeros)
        return out_arrs, out_names, out_avals

    return run


def kernel(z, w1, b1, w2, b2, gamma, beta):
    in_maps, w2_signs = host_precompute(z, w1, b1, w2, b2, gamma, beta)
    if w2_signs not in _CACHE:
        nc = build_graph(w2_signs)
        _CACHE[w2_signs] = _make_runner(nc)
    run = _CACHE[w2_signs]

    out_arrs, out_names, out_avals = run(in_maps)
    oi = out_names.index("out")
    full = np.asarray(out_arrs[oi]).reshape(NCORES, *out_avals[oi].shape)

    out = np.zeros((N, N), np.float32)
    for c in range(NCORES):
        for t, (bi, bj) in enumerate(core_tiles(c)):
            out[bi * TI : (bi + 1) * TI, bj * TJ : (bj + 1) * TJ] = full[c, t]
    iu = np.triu_indices(N, k=1)
    masked = np.zeros_like(out)
    masked[iu] = out[iu]
    return masked


# revision 12
# speedup vs baseline: 6.6511x; 6.6511x over previous
"""Trainium2 Bass kernel for nn_AltInnerProductDecoder.

Computes, for all upper-triangular pairs (i<j) of N=2048 nodes:
    f    = concat(z[i]*z[j], j-i)                  # 65 features
    y    = LayerNorm(f) * gamma + beta
    h    = relu(y @ w1 + b1)                       # 32 hidden
    out  = h @ w2 + b2                             # scalar per pair
scattered into a dense [N, N] upper-triangular adjacency.

Strategy: the per-pair 65->32 linear layer is a set of 32 weighted Gram
matrices of z, computed as dense [128 x 512] output tiles on the
TensorEngine (float32r, 1 cycle/row).  LayerNorm folds:
  out(i,j) = rstd * ( sum_k w2_k * relu( G'_k + b1'_k * u ) + b2 * u )
with u = std = sqrt(var+eps), rstd = 1/u (rstd*u = 1 kills b2's u).
G'_k is the mean-corrected Gram (correction folded into weights on host).
The distance feature d = j-i is folded into extra contraction rows using
bf16-exact hi/lo splits so reduced-precision matmul modes stay accurate.

8 cores, SPMD: 40 upper-triangle tiles of [128 x 512] split 5-per-core
(core c owns i-tile-rows c and 15-c).  All per-core variation is input
data (same graph).  Host assembles + masks with np.triu(k=1).
"""

import os
import sys

import numpy as np

sys.path.insert(0, "/opt/trn_rl_repo")

N = 2048
L = 64
D = 65  # input features to MLP
H = 32  # hidden
EPS = 1e-5
TI = 128  # i-tile rows
TJ = 512  # j-tile cols
NB_I = N // TI  # 16
NB_J = N // TJ  # 4
NCORES = 8
TILES_PER_CORE = 5

K1 = 72  # contraction rows for mov1/statG/statS0
K2 = 75  # contraction rows for mov2/statS1

# DVE handles k's with k%8<5 (20 of 32, fused relu+acc chain);
# ACT handles the rest (12, relu -> PE acc matmul).
DVE_KS = [k for k in range(H) if k % 8 < 5]
ACT_KS = [k for k in range(H) if k % 8 >= 5]


def core_tiles(c: int) -> list[tuple[int, int]]:
    """5 (bi, bj) tiles for core c covering the upper triangle."""
    tiles = []
    for bi in (c, NB_I - 1 - c):
        bj_min = (TI * bi + 1) // TJ
        for bj in range(bj_min, NB_J):
            tiles.append((bi, bj))
    assert len(tiles) == TILES_PER_CORE, (c, tiles)
    return tiles


def _bf16_round(x: np.ndarray) -> np.ndarray:
    import ml_dtypes

    return x.astype(np.float32).astype(ml_dtypes.bfloat16).astype(np.float64)


def _split3(x: np.ndarray):
    """3-way bf16-exact split: x ~= h + m + l with each part bf16-representable."""
    h = _bf16_round(x)
    r = x - h
    m = _bf16_round(r)
    l = _bf16_round(r - m)
    return h, m, l


def _split2(x: np.ndarray):
    h = _bf16_round(x)
    l = _bf16_round(x - h)
    return h, l


def host_precompute(z, w1, b1, w2, b2, gamma, beta):
    """Returns per-core input maps (list of dicts of np.float32 arrays) and
    the folded scalars needed at graph-build time."""
    z = np.asarray(z, np.float64)
    w1 = np.asarray(w1, np.float64)
    b1 = np.asarray(b1, np.float64).reshape(H)
    w2 = np.asarray(w2, np.float64).reshape(H)
    b2 = float(np.asarray(b2, np.float64).reshape(()))
    gamma = np.asarray(gamma, np.float64).reshape(D)
    beta = np.asarray(beta, np.float64).reshape(D)

    wg = gamma[:, None] * w1  # [65, 32]
    ck = wg.sum(axis=0)  # [32]
    wg2 = wg - ck[None, :] / D  # [65, 32] mean-corrected
    b1p = b1 + beta @ w1  # [32]

    sc = np.where(w2 >= 0, np.abs(w2), -np.abs(w2))  # = w2 (signed scale)

    z2 = z * z

    in_maps = []
    for c in range(NCORES):
        tiles = core_tiles(c)
        statG = np.zeros((TILES_PER_CORE, K1, H + 1, TI), np.float64)
        statS1 = np.zeros((TILES_PER_CORE, K2, TI), np.float64)
        mov1 = np.zeros((TILES_PER_CORE, K1, TJ), np.float64)
        mov2 = np.zeros((TILES_PER_CORE, K2, TJ), np.float64)

        for t, (bi, bj) in enumerate(tiles):
            i0, j0 = bi * TI, bj * TJ
            icc = np.arange(TI, dtype=np.float64) - (TI - 1) / 2.0  # [-63.5..63.5]
            jcc = np.arange(TJ, dtype=np.float64) - (TJ - 1) / 2.0  # [-255.5..255.5]
            D0 = (j0 + (TJ - 1) / 2.0) - (i0 + (TI - 1) / 2.0)  # d = D0 - icc + jcc
            r_i = D0 - icc  # [TI], d(i,j) = r_i + jcc[j]

            zi = z[i0 : i0 + TI]  # [TI, L]
            zj = z[j0 : j0 + TJ]  # [TJ, L]

            jh, jl = _split2(jcc)
            jsq3 = _split3(jcc * jcc)

            # ---- mov1 [K1, TJ]: rows 0-63 z_j; 64-66 ones; 67 jh; 68 jl;
            #      69 jh; 70-71 split2(jcc/65) (for S0, stat=1 exact)
            mov1[t, :L] = zj.T
            mov1[t, L : L + 3] = 1.0
            mov1[t, 67] = jh
            mov1[t, 68] = jl
            mov1[t, 69] = jh
            j65h, j65l = _split2(jcc / D)
            mov1[t, 70] = j65h
            mov1[t, 71] = j65l

            # ---- mov2 [K2, TJ]: rows 0-63 z_j^2; 64-66 ones; 67-69 jh;
            #      70-71 jl; 72-74 split3(jcc^2/65) (stat=1 exact)
            mov2[t, :L] = (zj * zj).T
            mov2[t, L : L + 3] = 1.0
            mov2[t, 67:70] = jh
            mov2[t, 70:72] = jl
            jsq65_3 = _split3(jcc * jcc / D)
            for r in range(3):
                mov2[t, 72 + r] = jsq65_3[r]

            # ---- statG for k in 0..31 (scaled by sc_k) and S0 at index 32
            for k in range(H):
                s = sc[k]
                statG[t, :L, k] = (zi * (wg2[:L, k] * s)).T  # [L, TI]
                q = (wg2[L, k] * s) * r_i  # [TI]
                q3 = _split3(q)
                for r in range(3):
                    statG[t, L + r, k] = q3[r]
                wh, wl = _split2(np.asarray(wg2[L, k] * s))
                statG[t, 67, k] = wh  # pairs mov1 jh
                statG[t, 68, k] = wh  # pairs mov1 jl
                statG[t, 69, k] = wl  # pairs mov1 jh
            # S0 (mean = m0/65): stationary scaled by 1/65
            statG[t, :L, H] = (zi / D).T
            r3 = _split3(r_i / D)
            for r in range(3):
                statG[t, L + r, H] = r3[r]
            statG[t, 70, H] = 1.0
            statG[t, 71, H] = 1.0

            # ---- statS1 (m1/65): rows 0-63 z_i^2/65; 64-66 r_i^2/65 splits;
            #      67-69 (2 r_i/65) 3-split x jh; 70-71 first two x jl; 72-74 1/65
            statS1[t, :L] = (zi * zi / D).T
            r2_3 = _split3(r_i * r_i / D)
            for r in range(3):
                statS1[t, L + r] = r2_3[r]
            s3 = _split3(2.0 * r_i / D)
            for r in range(3):
                statS1[t, 67 + r] = s3[r]
            statS1[t, 70] = s3[0]
            statS1[t, 71] = s3[1]
            statS1[t, 72:75] = 1.0

        # identities: k -> b1p[k]*sc[k]*I ; 32 -> b2*I ; 33 -> +I ; 34 -> -I
        ids = np.zeros((TI, H + 3, TI), np.float64)
        eye = np.eye(TI)
        for k in range(H):
            ids[:, k, :] = (b1p[k] * sc[k]) * eye
        ids[:, H, :] = b2 * eye
        ids[:, H + 1, :] = eye
        ids[:, H + 2, :] = -eye

        import ml_dtypes

        bf16 = ml_dtypes.bfloat16
        in_maps.append(
            {
                "statg": np.ascontiguousarray(statG.astype(np.float32), bf16).reshape(
                    TILES_PER_CORE, K1, (H + 1) * TI
                ),
                "stats1": np.ascontiguousarray(statS1.astype(np.float32), bf16),
                "mov1": np.ascontiguousarray(mov1.astype(np.float32), bf16),
                "mov2": np.ascontiguousarray(mov2.astype(np.float32), bf16),
                "ids": np.ascontiguousarray(ids.astype(np.float32), bf16),
            }
        )

    w2_signs = tuple(bool(s >= 0) for s in w2)
    return in_maps, w2_signs


def build_graph(w2_signs):
    """Build the SPMD Bacc graph (same for all cores)."""
    from concourse import bacc, mybir
    import concourse.bass as bass
    import concourse.tile as tile

    f32 = mybir.dt.float32
    bf16 = mybir.dt.bfloat16
    AF = mybir.ActivationFunctionType
    ALU = mybir.AluOpType

    nc = bacc.Bacc("TRN2", target_bir_lowering=False, debug=False, num_devices=NCORES)

    statg_d = nc.declare_dram_parameter(
        "statg", [TILES_PER_CORE, K1, (H + 1) * TI], bf16, isOutput=False
    )
    stats1_d = nc.declare_dram_parameter(
        "stats1", [TILES_PER_CORE, K2, TI], bf16, isOutput=False
    )
    mov1_d = nc.declare_dram_parameter(
        "mov1", [TILES_PER_CORE, K1, TJ], bf16, isOutput=False
    )
    mov2_d = nc.declare_dram_parameter(
        "mov2", [TILES_PER_CORE, K2, TJ], bf16, isOutput=False
    )
    ids_d = nc.declare_dram_parameter("ids", [TI, H + 3, TI], bf16, isOutput=False)
    out_d = nc.declare_dram_parameter(
        "out", [TILES_PER_CORE, TI, TJ], f32, isOutput=True
    )

    with tile.TileContext(nc) as tc:
        with (
            tc.tile_pool(name="consts", bufs=1) as consts,
            tc.tile_pool(name="statg", bufs=2) as statg_p,
            tc.tile_pool(name="stats1", bufs=2) as stats1_p,
            tc.tile_pool(name="mov", bufs=2) as mov_p,
            tc.tile_pool(name="chain", bufs=3) as chain_p,
            tc.tile_pool(name="work", bufs=2) as work_p,
            tc.tile_pool(name="hrelu", bufs=3) as hrelu_p,
            tc.tile_pool(name="ps_s0", bufs=2, space="PSUM") as ps_s0,
            tc.tile_pool(name="ps_s1", bufs=2, space="PSUM") as ps_s1,
            tc.tile_pool(name="ps_acc", bufs=2, space="PSUM") as ps_acc,
            tc.tile_pool(name="ps_g", bufs=2, space="PSUM") as ps_g,
        ):
            ids_sb = consts.tile([TI, H + 3, TI], bf16)
            nc.sync.dma_start(ids_sb[:], ids_d[:])
            eps_sb = consts.tile([TI, 1], f32)
            nc.vector.memset(eps_sb[:], EPS)

            state = {}

            def prologue(t):
                sg = statg_p.tile([K1, H + 1, TI], bf16, tag="sg")
                nc.sync.dma_start(sg[:], statg_d[t].rearrange("k (a p) -> k a p", p=TI))
                s1t = stats1_p.tile([K2, TI], bf16, tag="s1t")
                nc.sync.dma_start(s1t[:], stats1_d[t])
                m1 = mov_p.tile([K1, TJ], bf16, tag="m1")
                nc.sync.dma_start(m1[:], mov1_d[t])
                m2 = mov_p.tile([K2, TJ], bf16, tag="m2")
                nc.sync.dma_start(m2[:], mov2_d[t])

                s0_ps = ps_s0.tile([TI, TJ], f32, tag="s0ps")
                s1_ps = ps_s1.tile([TI, TJ], f32, tag="s1ps")
                nc.tensor.matmul(s0_ps[:], sg[:, H, :], m1[:])
                nc.tensor.matmul(s1_ps[:], s1t[:], m2[:])

                mean_sb = work_p.tile([TI, TJ], f32, tag="mean")
                nc.scalar.copy(mean_sb[:], s0_ps[:])
                q = work_p.tile([TI, TJ], f32, tag="q")
                nc.vector.tensor_mul(q[:], mean_sb[:], mean_sb[:])  # mean^2
                var = work_p.tile([TI, TJ], f32, tag="var")
                nc.vector.tensor_sub(var[:], s1_ps[:], q[:])
                lnv = work_p.tile([TI, TJ], f32, tag="lnv")
                nc.scalar.activation(lnv[:], var[:], AF.Ln, bias=eps_sb[:])
                u = work_p.tile([TI, TJ], bf16, tag="u")
                nc.scalar.activation(u[:], lnv[:], AF.Exp, scale=0.5)
                rstd = work_p.tile([TI, TJ], f32, tag="rstd")
                nc.scalar.activation(rstd[:], lnv[:], AF.Exp, scale=-0.5)
                state[t] = (sg, m1, u, rstd)

            def kloop(t):
                sg, m1, u, rstd = state.pop(t)
                acc_ps = ps_acc.tile([TI, TJ], f32, tag="accps")
                # ACC = b2 * u
                nc.tensor.matmul(
                    acc_ps[:], ids_sb[:, H, :], u[:], start=True, stop=False
                )
                chain_prev = None
                n_act_done = 0
                for k in range(H):
                    g_ps = ps_g.tile([TI, TJ], f32, tag="gps")
                    nc.tensor.matmul(
                        g_ps[:], sg[:, k, :], m1[:], start=True, stop=False
                    )
                    nc.tensor.matmul(
                        g_ps[:], ids_sb[:, k, :], u[:], start=False, stop=True
                    )
                    pos = w2_signs[k]
                    if k % 8 < 5:  # DVE: fused relu + accumulate chain
                        op0 = ALU.max if pos else ALU.min
                        cnew = chain_p.tile([TI, TJ], f32, tag="chain")
                        if chain_prev is None:
                            nc.vector.tensor_scalar(
                                cnew[:], g_ps[:], 0.0, None, op0
                            )
                        else:
                            nc.vector.scalar_tensor_tensor(
                                cnew[:], g_ps[:], 0.0, chain_prev[:], op0, ALU.add
                            )
                        chain_prev = cnew
                    else:  # ACT relu -> PE accumulate
                        h = hrelu_p.tile([TI, TJ], bf16, tag="h")
                        nc.scalar.activation(
                            h[:], g_ps[:], AF.Relu, scale=1.0 if pos else -1.0
                        )
                        n_act_done += 1
                        last = n_act_done == len(ACT_KS)
                        idm = H + 1 if pos else H + 2
                        nc.tensor.matmul(
                            acc_ps[:],
                            ids_sb[:, idm, :],
                            h[:],
                            start=False,
                            stop=last,
                        )
                # out = rstd * (ACC + chain)
                tsum = work_p.tile([TI, TJ], f32, tag="tsum")
                nc.vector.scalar_tensor_tensor(
                    tsum[:], acc_ps[:], 1.0, chain_prev[:], ALU.mult, ALU.add
                )
                out_sb = work_p.tile([TI, TJ], f32, tag="outsb")
                nc.vector.tensor_mul(out_sb[:], tsum[:], rstd[:])
                nc.sync.dma_start(out_d[t], out_sb[:])

            prologue(0)
            for t in range(TILES_PER_CORE):
                if t + 1 < TILES_PER_CORE:
                    prologue(t + 1)
                kloop(t)

    nc.compile()
    return nc


_CACHE = {}


def _make_runner(nc):
    """Build a cached jitted SPMD runner (mirrors bass2jax.run_bass_via_pjrt
    but keeps the compiled executable across calls)."""
    import jax
    from jax.experimental.shard_map import shard_map
    from jax.sharding import Mesh, PartitionSpec
    from concourse import bass2jax as b2j
    from concourse import mybir

    b2j.install_neuronx_cc_hook()

    partition_name = nc.partition_id_tensor.name if nc.partition_id_tensor else None
    in_names, out_names, out_avals, zero_outs = [], [], [], []
    for alloc in nc.m.functions[0].allocations:
        if not isinstance(alloc, mybir.MemoryLocationSet):
            continue
        name = alloc.memorylocations[0].name
        if alloc.kind == "ExternalInput":
            if name != partition_name:
                in_names.append(name)
        elif alloc.kind == "ExternalOutput":
            out_names.append(name)
            shape = tuple(alloc.tensor_shape)
            dtype = mybir.dt.np(alloc.dtype)
            out_avals.append(jax.core.ShapedArray(shape, dtype))
            zero_outs.append(np.zeros(shape, dtype))
    n_params = len(in_names)
    all_in_names = list(in_names) + list(out_names)
    if partition_name is not None:
        all_in_names.append(partition_name)

    def _body(*args):
        operands = list(args)
        if partition_name is not None:
            operands.append(b2j.partition_id_tensor())
        outs = b2j._bass_exec_p.bind(
            *operands,
            out_avals=tuple(out_avals),
            in_names=tuple(all_in_names),
            out_names=tuple(out_names),
            lowering_input_output_aliases=(),
            sim_require_finite=False,
            sim_require_nnan=False,
            nc=nc,
        )
        return tuple(outs)

    devices = jax.devices()[:NCORES]
    mesh = Mesh(np.asarray(devices), ("core",))
    n_outs = len(out_names)
    in_specs = (PartitionSpec("core"),) * (n_params + n_outs)
    out_specs = (PartitionSpec("core"),) * n_outs
    sharded = jax.jit(
        shard_map(
            _body, mesh=mesh, in_specs=in_specs, out_specs=out_specs, check_rep=False
        ),
        keep_unused=True,
    )
    concat_zeros = [
        np.zeros((NCORES * z.shape[0], *z.shape[1:]), z.dtype) for z in zero_outs
    ]

    def concat_inputs(in_maps):
        return [
            np.concatenate([in_maps[c][name] for c in range(NCORES)], axis=0)
            for name in in_names
        ]

    def device_put_inputs(in_maps):
        from jax.sharding import NamedSharding

        sh = NamedSharding(mesh, PartitionSpec("core"))
        return [jax.device_put(a, sh) for a in concat_inputs(in_maps)] + [
            jax.device_put(z, sh) for z in concat_zeros
        ]

    def run(in_maps):
        out_arrs = sharded(*concat_inputs(in_maps), *concat_zeros)
        return out_arrs, out_names, out_avals

    run.sharded = sharded
    run.device_put_inputs = device_put_inputs
    run.out_names = out_names
    run.out_avals = out_avals
    return run


def kernel(z, w1, b1, w2, b2, gamma, beta):
    in_maps, w2_signs = host_precompute(z, w1, b1, w2, b2, gamma, beta)
    if w2_signs not in _CACHE:
        nc = build_graph(w2_signs)
        _CACHE[w2_signs] = _make_runner(nc)
    run = _CACHE[w2_signs]

    out_arrs, out_names, out_avals = run(in_maps)
    oi = out_names.index("out")
    full = np.asarray(out_arrs[oi]).reshape(NCORES, *out_avals[oi].shape)

    out = np.zeros((N, N), np.float32)
    for c in range(NCORES):
        for t, (bi, bj) in enumerate(core_tiles(c)):
            out[bi * TI : (bi + 1) * TI, bj * TJ : (bj + 1) * TJ] = full[c, t]
    iu = np.triu_indices(N, k=1)
    masked = np.zeros_like(out)
    masked[iu] = out[iu]
    return masked
